# revision 3
# baseline (speedup 1.0000x reference)
import math
import sys

import numpy as np

sys.path.insert(0, "/opt/trn_rl_repo")

import concourse.bass as bass  # noqa: E402
import concourse.tile as tile  # noqa: E402
from concourse import bacc, mybir  # noqa: E402
from concourse.bass_utils import run_bass_kernel_spmd  # noqa: E402

# Problem constants (hardcoded per spec)
B = 4
D = 2048
L = 2048
N = 16
NCORES = 8
DLOC = D // NCORES  # 256 channels per core
C = 128             # chunk length / conv band width
NCH = L // C        # 16 chunks
KLEN = 2 * C        # conv kernel lags used: 0..255
G = 4               # channels per DMA group
NG = DLOC // G      # 64 groups per core

F32 = mybir.dt.float32

TRACE = False
LAST_EXEC_NS = None
_NC = None


def _sigmoid(v):
    return 1.0 / (1.0 + np.exp(-v))


def _build_nc():
    nc = bacc.Bacc(None, target_bir_lowering=False, debug=False)
    x_d = nc.declare_dram_parameter("x", (NG, C, G, B, NCH + 1), F32, isOutput=False)
    w_d = nc.declare_dram_parameter("w", (NG, C, G, 2, C), F32, isOutput=False)
    o_d = nc.declare_dram_parameter("out", (NG, C, G, B, NCH), F32, isOutput=True)

    with tile.TileContext(nc) as tc:
        with (
            tc.tile_pool(name="xp", bufs=3) as xp,
            tc.tile_pool(name="wp", bufs=3) as wp,
            tc.tile_pool(name="pp", bufs=8, space="PSUM") as pp,
            tc.tile_pool(name="op", bufs=4) as op,
        ):
            for gi in range(NG):
                xt = xp.tile([C, G, B, NCH + 1], F32, tag="x")
                nc.sync.dma_start(xt[:], x_d[gi])
                wt = wp.tile([C, G, 2, C], F32, tag="w")
                nc.sync.dma_start(wt[:], w_d[gi])
                ot = op.tile([C, G, B, NCH], F32, tag="o")
                for gj in range(G):
                    pt = pp.tile([C, B, NCH], F32, tag="p")
                    # y_chunk = T0^T @ x_chunk + T1^T @ x_prev_chunk
                    nc.tensor.matmul(
                        pt[:], wt[:, gj, 0, :], xt[:, gj, :, 1:],
                        start=True, stop=False,
                    )
                    nc.tensor.matmul(
                        pt[:], wt[:, gj, 1, :], xt[:, gj, :, 0:NCH],
                        start=False, stop=True,
                    )
                    nc.any.tensor_copy(ot[:, gj], pt[:])
                nc.sync.dma_start(o_d[gi], ot[:])
    nc.compile()
    return nc


def _get_nc():
    global _NC
    if _NC is None:
        _NC = _build_nc()
    return _NC


def kernel(x, alpha, delta, theta, gamma, omega):
    global LAST_EXEC_NS
    x = np.asarray(x, np.float32)
    alpha = np.asarray(alpha, np.float64)
    delta = np.asarray(delta, np.float64)
    theta = np.asarray(theta, np.float64)
    gamma = np.asarray(gamma, np.float64)
    omega = np.asarray(omega, np.float64)

    # --- host: conv-kernel coefficients (tiny: O(D*N*KLEN)) ---
    p = _sigmoid(alpha[..., 0])             # (D, N)
    dd = _sigmoid(delta[..., 0])            # (D, N)
    wave = np.arange(1, N + 1, dtype=np.float64)
    phi = wave[None, :] * (_sigmoid(theta[:, 0, 0])[:, None] * (2.0 * math.pi / N))
    q = (1.0 - p * dd) * np.exp(1j * phi)   # (D, N) complex
    g = (gamma[..., 0] + 1j * gamma[..., 1]) * math.sqrt(1.0 / N)
    coef = g * p                            # (D, N)
    Q = q[:, :, None] ** np.arange(KLEN)[None, None, :]   # (D, N, KLEN)
    kk = np.real(np.einsum("dn,dnt->dt", coef, Q))        # (D, KLEN)
    kk[:, 0] += omega

    # banded Toeplitz blocks: T0 lower-tri (lags 0..C-1), T1 dense (lags 1..2C-1)
    lag = np.arange(C)[None, :] - np.arange(C)[:, None]   # (s, r) = r - s
    T0 = np.where(lag >= 0, kk[:, np.clip(lag, 0, None)], 0.0)  # (D, C, C)
    T1 = kk[:, C + lag]                                         # (D, C, C)
    w = np.stack([T0, T1], axis=2).astype(np.float32)           # (D, s, 2, r)
    w = np.ascontiguousarray(
        w.reshape(NCORES, NG, G, C, 2, C).transpose(0, 1, 3, 2, 4, 5)
    )  # (cores, NG, C, G, 2, C)

    # x layout: (d, s, b, j+1) with a zero chunk-column at j=0
    xr = x.reshape(B, D, NCH, C).transpose(1, 3, 0, 2)    # (D, C, B, NCH)
    xs = np.zeros((D, C, B, NCH + 1), np.float32)
    xs[:, :, :, 1:] = xr
    xs = np.ascontiguousarray(
        xs.reshape(NCORES, NG, G, C, B, NCH + 1).transpose(0, 1, 3, 2, 4, 5)
    )  # (cores, NG, C, G, B, NCH+1)

    in_maps = [{"x": xs[i], "w": w[i]} for i in range(NCORES)]
    nc = _get_nc()
    try:
        res = run_bass_kernel_spmd(
            nc, in_maps, core_ids=list(range(NCORES)), trace=TRACE
        )
    except Exception:
        if not TRACE:
            raise
        res = run_bass_kernel_spmd(nc, in_maps, core_ids=list(range(NCORES)))
    LAST_EXEC_NS = getattr(res, "exec_time_ns", None)

    out = np.stack([res.results[i]["out"] for i in range(NCORES)], axis=0)
    # (cores, NG, C, G, B, NCH) -> (D, C, B, NCH)
    out = out.transpose(0, 1, 3, 2, 4, 5).reshape(D, C, B, NCH)
    y = out.transpose(2, 0, 3, 1).reshape(B, D, L)
    return y.astype(np.float32)
